# revision 16
# baseline (speedup 1.0000x reference)
"""Trainium2 Bass kernel: DifferentiableAddressingHead (NTM-style addressing).

Sharding: pure data parallelism over the batch axis (64 of 512 batch rows
per core); the tiny dense weights are replicated. No collectives.

v2 design (PE-centric stage B):
  The baseline was DVE-bound: the per-(b,m) dot products and row-norms were
  a multiply + grouped tensor_reduce on the vector engine (~417us).  Here
  the host repacks memory as *bf16, transposed*: memT[b] is a [128, 2048]
  tile with partitions = (m-parity, d) and free = m//2.  That makes the
  d-contraction a partition-dim contraction, which the TensorEngine does at
  1 col/cycle:
    - dot[b, m]   = q[b]^T @ memT[b]   (stationary = q packed at the
      batch's column pair, zeros elsewhere)
    - normsq[b,m] = ones^T @ (memT[b]^2)  (squares on ACT/DVE, bf16 2x)
  16 batches accumulate into one [32, 512] PSUM region (zero stationary
  columns contribute nothing), so PSUM fills densely: one [128, 512] bank
  per m-chunk holds dots+normsq for 32 batches.  8 banks = 64 batches.
  PSUM -> SBUF copies are 8 x [128, 512], then 64 reorg DMAs assemble
  d_dot/d_nsq as [64 batch, (parity, m//2)] tiles.

  Stage C (softmax/gate/3-tap circular conv/sharpen) runs in the parity
  layout: the conv's m+-1 taps become free-dim shifts between the parity
  halves.  The final normalize writes with a stride-2 free AP to restore
  natural m-order before the output DMA.  bf16 memory costs ~6e-4 rel
  error (tolerance 2e-2).
"""

from contextlib import ExitStack

import numpy as np

import concourse.bass as bass
import concourse.tile as tile
from concourse import masks, mybir

B, M, D, C = 512, 4096, 64, 256
NCORES = 8
BL = B // NCORES  # 64 batch rows per core
NSHIFT = 3
EPS = 1e-8

F32 = mybir.dt.float32
BF16 = mybir.dt.bfloat16
AF = mybir.ActivationFunctionType
ALU = mybir.AluOpType
AX = mybir.AxisListType

P = 128            # SBUF partitions
F2 = M // 2        # 2048 free elements (m//2) in the transposed layout
CH = 512           # psum chunk (one bank of f32)
NCH = F2 // CH     # 4 chunks


def _body(tc, nc, memT, cs, prev, Wk, bb, wheads, out):
    ctx = tc._body_ctx

    const = ctx.enter_context(tc.tile_pool(name="const", bufs=1))
    small = ctx.enter_context(tc.tile_pool(name="small", bufs=1))
    mem_pool = ctx.enter_context(tc.tile_pool(name="mem", bufs=6))
    sq_pool = ctx.enter_context(tc.tile_pool(name="sq", bufs=4))
    stage_pool = ctx.enter_context(tc.tile_pool(name="stg", bufs=1))
    big = ctx.enter_context(tc.tile_pool(name="big", bufs=1))

    # ---------------- constants ----------------
    ident = const.tile([BL, BL], F32, tag="ident")
    masks.make_identity(nc, ident[:])
    ones_row = const.tile([1, P], F32, tag="ones")
    nc.gpsimd.memset(ones_row[:], 1.0)

    # qpk: per-batch stationary [128, 64] blocks.  Block b has q[b] (bf16)
    # at column (b%32) on partitions 0:64 and column (b%32)+32 on 64:128,
    # zeros elsewhere.  onespk: the same for the ones vectors (32 distinct
    # blocks indexed by b%32).
    qpk = const.tile([P, 64 * BL], BF16, tag="qpk")
    nc.gpsimd.memset(qpk[:], 0.0)
    onespk = const.tile([P, 64 * 32], BF16, tag="onespk")
    nc.gpsimd.memset(onespk[:], 0.0)
    # ones at global col 65*r (partitions 0:64) / 65*r+32 (64:128)
    nc.gpsimd.memset(onespk[0:64, 0:2016:65], 1.0)
    nc.gpsimd.memset(onespk[64:128, 32:2048:65], 1.0)

    # ---------------- load controller + host-prepacked weights ----------
    cs_sb = small.tile([BL, C], F32, tag="cs")
    nc.sync.dma_start(cs_sb[:], cs[:])
    wk_sb = small.tile([P, 2 * D], F32, tag="wk")
    nc.sync.dma_start(wk_sb[:], Wk[:])           # wk_pack [128, 128]
    wh_sb = small.tile([P, 12], F32, tag="wh")
    nc.sync.dma_start(wh_sb[:], wheads[:])       # wh_pack [128, 12]
    brow = small.tile([1, 6], F32, tag="brow")
    nc.sync.dma_start(brow[:], bb[:])            # b_pack [1, 6]

    # prev loaded early (staged into d_nsq's buffer, which is dead until
    # the first reorg DMA); split+gate-scaled once omg is ready
    d_dot = big.tile([BL, M], F32, tag="ddot")
    d_nsq = big.tile([BL, M], F32, tag="dnsq")
    prev_nat = d_nsq[:, :]
    nc.gpsimd.dma_start(prev_nat, prev[:])

    # ---------------- stage A: controller projections ----------------
    with tc.tile_pool(name="spsum", bufs=2, space="PSUM") as spsum:
        # transpose cs -> csT [C(2x128 part), BL]
        csT = small.tile([P, 2 * BL], F32, tag="csT")
        for ci in range(2):
            t_ps = spsum.tile([P, BL], F32, tag="tps")
            nc.tensor.transpose(t_ps[:], cs_sb[:, ci * P:(ci + 1) * P], ident[:])
            nc.vector.tensor_copy(csT[:, ci * BL:(ci + 1) * BL], t_ps[:])

        # query + heads on PE
        q_ps = spsum.tile([BL, D], F32, tag="qps")
        nc.tensor.matmul(q_ps[:], csT[:, 0:BL], wk_sb[:, 0:D],
                         start=True, stop=False)
        nc.tensor.matmul(q_ps[:], csT[:, BL:2 * BL], wk_sb[:, D:2 * D],
                         start=False, stop=True)
        q_sb = small.tile([BL, D], F32, tag="qsb")
        nc.vector.tensor_copy(q_sb[:], q_ps[:])

        h_ps = spsum.tile([BL, 6], F32, tag="hps")
        nc.tensor.matmul(h_ps[:], csT[:, 0:BL], wh_sb[:, 0:6],
                         start=True, stop=False)
        nc.tensor.matmul(h_ps[:], csT[:, BL:2 * BL], wh_sb[:, 6:12],
                         start=False, stop=False)
        nc.tensor.matmul(h_ps[:], ones_row[0:1, 0:BL], brow[:],
                         start=False, stop=True)
        h_sb = small.tile([BL, 6], F32, tag="hsb")
        nc.vector.tensor_copy(h_sb[:], h_ps[:])

        # qT: q transposed to [d-part, b-free], duplicated on both
        # partition halves for the stationary packs.  Transpose outputs
        # must start at PSUM partition 0, so duplicate q along the free
        # dim first and transpose [64, 128] -> [128, 64] in one shot.
        q2 = small.tile([BL, P], F32, tag="q2")
        nc.vector.tensor_copy(q2[:, 0:D], q_sb[:])
        nc.vector.tensor_copy(q2[:, D:P], q_sb[:])
        qT_ps = spsum.tile([P, BL], F32, tag="qTps")
        nc.tensor.transpose(qT_ps[:], q2[:], ident[:])
        # scatter into qpk: dest col of batch b=32j+r is 2048j + 65r (+32
        # on the lower partition half); view blocks of 2048, step by 65.
        qpk_v = qpk[:].rearrange("p (j r) -> p j r", j=2)
        qsrc = qT_ps[:].rearrange("p (j r) -> p j r", j=2)
        nc.vector.tensor_copy(qpk_v[0:D, :, 0:2016:65], qsrc[0:D, :, :])
        nc.vector.tensor_copy(qpk_v[D:P, :, 32:2048:65], qsrc[D:P, :, :])

    # ---------------- per-batch scalars ----------------
    qsq = small.tile([BL, D], F32, tag="qsq")
    qn2 = small.tile([BL, 1], F32, tag="qn2")
    nc.scalar.activation(qsq[:], q_sb[:], AF.Square, accum_out=qn2[:])
    qnorm = small.tile([BL, 1], F32, tag="qnorm")
    nc.scalar.activation(qnorm[:], qn2[:], AF.Sqrt)
    qne = small.tile([BL, 1], F32, tag="qne")
    nc.vector.tensor_scalar(qne[:], qnorm[:], EPS, None, op0=ALU.add)
    qrecip = small.tile([BL, 1], F32, tag="qrecip")
    nc.vector.reciprocal(qrecip[:], qne[:])

    # bscale = (softplus(h0)+1) / (|q|+eps); folded into the cosine-sim
    # rsqrt via ln(bscale) as an Exp bias later.
    spe = small.tile([BL, 1], F32, tag="spe")
    nc.scalar.activation(spe[:], h_sb[:, 0:1], AF.Exp)
    spb = small.tile([BL, 1], F32, tag="spb")
    nc.scalar.activation(spb[:], spe[:], AF.Ln, bias=1.0)
    bscale = small.tile([BL, 1], F32, tag="bscale")
    nc.vector.tensor_scalar(bscale[:], spb[:], 1.0, qrecip[:],
                            op0=ALU.add, op1=ALU.mult)
    lnbsc = small.tile([BL, 1], F32, tag="lnbsc")
    nc.scalar.activation(lnbsc[:], bscale[:], AF.Ln)

    g_t = small.tile([BL, 1], F32, tag="gate")
    nc.scalar.activation(g_t[:], h_sb[:, 1:2], AF.Sigmoid)
    omg = small.tile([BL, 1], F32, tag="omg")
    nc.scalar.activation(omg[:], g_t[:], AF.Copy, bias=1.0, scale=-1.0)

    e3 = small.tile([BL, NSHIFT], F32, tag="e3")
    nc.scalar.activation(e3[:], h_sb[:, 2:5], AF.Exp)
    ssum = small.tile([BL, 1], F32, tag="ssum")
    nc.vector.tensor_reduce(ssum[:], e3[:], axis=AX.X, op=ALU.add)
    srec = small.tile([BL, 1], F32, tag="srec")
    nc.vector.reciprocal(srec[:], ssum[:])
    sk = small.tile([BL, NSHIFT], F32, tag="sk")
    nc.vector.tensor_scalar(sk[:], e3[:], srec[:], None, op0=ALU.mult)

    gse = small.tile([BL, 1], F32, tag="gse")
    nc.scalar.activation(gse[:], h_sb[:, 5:6], AF.Exp)
    gsp = small.tile([BL, 1], F32, tag="gsp")
    nc.scalar.activation(gsp[:], gse[:], AF.Ln, bias=1.0)
    gamma = small.tile([BL, 1], F32, tag="gamma")
    nc.vector.tensor_scalar(gamma[:], gsp[:], 1.0, None, op0=ALU.add)

    # prev split to parity layout, pre-scaled by (1-gate):
    # prev_t[b, e*F2 + F] = (1-gate[b]) * prev[b, 2F+e]
    prev_t = big.tile([BL, M], BF16, tag="prevt")
    for e in range(2):
        nc.scalar.activation(prev_t[:, e * F2:(e + 1) * F2],
                             prev_nat[:, e:M:2], AF.Copy, scale=omg[:])

    # ---------------- stage B: dots + norms on the PE ----------------
    mm_psum = ctx.enter_context(tc.tile_pool(name="mmps", bufs=1, space="PSUM"))

    # The nsq matmuls run one batch behind the dot matmuls so the PE never
    # waits on the just-computed square (any PE gap resets the DVFS ramp
    # and drops the array from 2.4 to 1.2 GHz).
    psum_c = {h: [mm_psum.tile([P, CH], F32, name=f"ps{h}{c}", tag=f"ps{h}{c}")
                  for c in range(NCH)] for h in range(2)}
    sq_tiles = {}

    def drain_half(h):
        rows = slice(32 * h, 32 * h + 32)
        for c in range(NCH):
            stg = stage_pool.tile([P, CH], F32, name=f"stg{h}{c}",
                                  tag=f"stg{h}{c}")
            if c % 2 == 0:
                nc.vector.tensor_copy(stg[:], psum_c[h][c][:])
            else:
                nc.scalar.copy(stg[:], psum_c[h][c][:])
            for e in range(2):
                dst = slice(e * F2 + CH * c, e * F2 + CH * (c + 1))
                nc.sync.dma_start(d_dot[rows, dst], stg[32 * e:32 * e + 32, :])
                nc.sync.dma_start(d_nsq[rows, dst],
                                  stg[64 + 32 * e:96 + 32 * e, :])

    def emit_nsq(b):
        h, bl = b // 32, b % 32
        sq = sq_tiles.pop(b)
        os_ = onespk[:, 64 * bl:64 * bl + 64]
        for c in range(NCH):
            nc.tensor.matmul(psum_c[h][c][64:128, :],
                             os_, sq[:, CH * c:CH * (c + 1)],
                             start=(bl == 0), stop=(bl == 31))
        if bl == 31:
            drain_half(h)

    for b in range(BL):
        h, bl = b // 32, b % 32
        mt = mem_pool.tile([P, F2], BF16, tag="memt")
        dma = nc.sync if (b % 2 == 0) else nc.gpsimd
        dma.dma_start(mt[:], memT[b])

        sq = sq_pool.tile([P, F2], BF16, tag="sq")
        if b % 2 == 0:
            nc.scalar.activation(sq[:], mt[:], AF.Square)
        else:
            nc.vector.tensor_tensor(sq[:], mt[:], mt[:], op=ALU.mult)
        sq_tiles[b] = sq

        qs = qpk[:, 64 * b:64 * b + 64]
        for c in range(NCH):
            nc.tensor.matmul(psum_c[h][c][0:64, :],
                             qs, mt[:, CH * c:CH * (c + 1)],
                             start=(bl == 0), stop=(bl == 31))
        if b >= 1:
            emit_nsq(b - 1)
    emit_nsq(BL - 1)

    # ---------------- stage C: postprocessing in row blocks ----------------
    RB = 32
    esum_a = small.tile([BL, 2], F32, tag="esum")
    erec_a = small.tile([BL, 1], F32, tag="erec")
    galpha_a = small.tile([BL, 1], F32, tag="galpha")
    psm_a = small.tile([BL, 2], F32, tag="psm")
    psme_a = small.tile([BL, 1], F32, tag="psme")
    prc_a = small.tile([BL, 1], F32, tag="prc")

    EH = slice(0, F2)        # even-m block (m = 2F)
    OH = slice(F2, M)        # odd-m block (m = 2F+1)
    for r0 in range(0, BL, RB):
        rows = slice(r0, r0 + RB)
        dd = d_dot[rows, :]
        dn = d_nsq[rows, :]
        halves = [EH, OH]

        # sim = dot * bscale * rsqrt(nsq); rsqrt+bscale via exp(-.5ln+lnb)
        for hh in halves:
            nc.scalar.activation(dn[:, hh], dn[:, hh], AF.Ln)
        for hh in halves:
            nc.scalar.activation(dn[:, hh], dn[:, hh], AF.Exp,
                                 scale=-0.5, bias=lnbsc[rows, :])
        for hh in halves:
            nc.vector.tensor_tensor(dd[:, hh], dd[:, hh], dn[:, hh],
                                    op=ALU.mult)
            # softmax numerator (logits bounded by beta: no max subtraction)
            nc.scalar.activation(dd[:, hh], dd[:, hh], AF.Exp)
        for i, hh in enumerate(halves):
            nc.vector.tensor_reduce(esum_a[rows, i:i + 1], dd[:, hh],
                                    axis=AX.X, op=ALU.add)
        erec = erec_a[rows, :]
        nc.vector.tensor_reduce(erec, esum_a[rows, :], axis=AX.X, op=ALU.add)
        nc.vector.reciprocal(erec, erec)
        galpha = galpha_a[rows, :]
        nc.vector.tensor_tensor(galpha, g_t[rows, :], erec, op=ALU.mult)

        # gated = galpha*exp + (1-gate)*prev   (in place in d_dot rows)
        pt = prev_t[rows, :]
        for hh in halves:
            nc.vector.scalar_tensor_tensor(dd[:, hh], dd[:, hh],
                                           galpha, pt[:, hh],
                                           op0=ALU.mult, op1=ALU.add)

        # 3-tap circular conv in parity layout, into the dead d_nsq rows.
        # shifted[m] = sk0*g[m-1] + sk1*g[m] + sk2*g[m+1]
        cv = dn
        sk0, sk1, sk2 = sk[rows, 0:1], sk[rows, 1:2], sk[rows, 2:3]
        # even block: g[m-1] -> odd[F-1], g[m+1] -> odd[F]
        nc.scalar.activation(cv[:, EH], dd[:, EH], AF.Copy, scale=sk1)
        nc.vector.scalar_tensor_tensor(cv[:, 1:F2], dd[:, F2:M - 1], sk0,
                                       cv[:, 1:F2], op0=ALU.mult, op1=ALU.add)
        nc.vector.scalar_tensor_tensor(cv[:, 0:1], dd[:, M - 1:M], sk0,
                                       cv[:, 0:1], op0=ALU.mult, op1=ALU.add)
        nc.vector.scalar_tensor_tensor(cv[:, EH], dd[:, OH], sk2,
                                       cv[:, EH], op0=ALU.mult, op1=ALU.add)
        # odd block: g[m-1] -> even[F], g[m+1] -> even[F+1]
        nc.scalar.activation(cv[:, OH], dd[:, OH], AF.Copy, scale=sk1)
        nc.vector.scalar_tensor_tensor(cv[:, OH], dd[:, EH], sk0,
                                       cv[:, OH], op0=ALU.mult, op1=ALU.add)
        nc.vector.scalar_tensor_tensor(cv[:, F2:M - 1], dd[:, 1:F2], sk2,
                                       cv[:, F2:M - 1],
                                       op0=ALU.mult, op1=ALU.add)
        nc.vector.scalar_tensor_tensor(cv[:, M - 1:M], dd[:, 0:1], sk2,
                                       cv[:, M - 1:M],
                                       op0=ALU.mult, op1=ALU.add)

        # sharpen: (conv+eps)^gamma = exp(gamma*ln(conv+eps)), normalize
        for hh in halves:
            nc.scalar.activation(cv[:, hh], cv[:, hh], AF.Ln, bias=EPS)
        for hh in halves:
            nc.scalar.activation(cv[:, hh], cv[:, hh], AF.Exp,
                                 scale=gamma[rows, :])
        for i, hh in enumerate(halves):
            nc.vector.tensor_reduce(psm_a[rows, i:i + 1], cv[:, hh],
                                    axis=AX.X, op=ALU.add)
        psme = psme_a[rows, :]
        nc.vector.tensor_reduce(psme, psm_a[rows, :], axis=AX.X, op=ALU.add)
        prc = prc_a[rows, :]
        nc.vector.tensor_scalar(psme, psme, EPS, None, op0=ALU.add)
        nc.vector.reciprocal(prc, psme)
        # final scale writes de-interleaved (stride-2) to restore m-order,
        # into the dead gated rows (d_dot)
        for e in range(2):
            nc.scalar.activation(dd[:, e:M:2],
                                 cv[:, e * F2:(e + 1) * F2], AF.Copy,
                                 scale=prc)
        nc.gpsimd.dma_start(out[rows, :], dd)


def build(split_waits=True):
    nc = bass.Bass()
    memT = nc.dram_tensor("memT", [BL, P, F2], BF16, kind="ExternalInput")
    cs = nc.dram_tensor("controller_state", [BL, C], F32, kind="ExternalInput")
    prev = nc.dram_tensor("previous_weights", [BL, M], F32, kind="ExternalInput")
    wk_pack = nc.dram_tensor("wk_pack", [P, 2 * D], F32, kind="ExternalInput")
    wh_pack = nc.dram_tensor("wh_pack", [P, 12], F32, kind="ExternalInput")
    b_pack = nc.dram_tensor("b_pack", [1, 6], F32, kind="ExternalInput")
    out = nc.dram_tensor("out", [BL, M], F32, kind="ExternalOutput")

    # register EPS so float biases on ACT instructions resolve to a const AP
    eps_t = nc.alloc_sbuf_tensor("const-f32-eps", [128, 1], F32)
    nc.gpsimd.memset(eps_t.ap(), EPS)
    nc.const_aps.aps[(F32, EPS)] = eps_t.ap()
    nc.all_engine_barrier()

    with tile.TileContext(nc) as tc:
        with ExitStack() as ctx:
            tc._body_ctx = ctx
            _body(tc, nc, memT, cs, prev, wk_pack, b_pack, wh_pack, out)
    if split_waits:
        _split_multiwait(nc)
    return nc


def _split_multiwait(nc, max_waits=1):
    """Walrus ISA structs encode a limited number of semaphore waits per
    instruction. Move all but one wait of any multi-wait instruction onto
    same-engine InstNoOp instructions inserted directly before it."""
    for fn in nc.m.functions:
        for blk in fn.blocks:
            insts = blk.instructions
            idx = 0
            while idx < len(insts):
                inst = insts[idx]
                si = inst.sync_info
                if si is not None and len(si.on_wait) > max_waits:
                    waits = list(si.on_wait)
                    extra, keep = waits[:-max_waits], waits[-max_waits:]
                    for w in extra:
                        nop = mybir.InstNoOp(
                            name=nc.get_next_instruction_name(),
                            sync_info=mybir.SyncInfo(on_wait=[w], on_update=[]),
                            bass_nofuse=True,
                            engine=inst.engine,
                        )
                        insts.insert(idx, nop)
                        idx += 1
                    inst.sync_info = mybir.SyncInfo(
                        on_wait=keep, on_update=list(si.on_update))
                idx += 1


_NC = None


def _get_nc():
    global _NC
    if _NC is None:
        _NC = build()
    return _NC


def _make_in_maps(inputs):
    import ml_dtypes
    full = {k: np.ascontiguousarray(np.asarray(v, dtype=np.float32))
            for k, v in inputs.items()}
    # memory -> bf16, transposed to [b, (parity, d), m//2]
    mem = full["memory"].astype(ml_dtypes.bfloat16)          # [B, M, D]
    memT = np.ascontiguousarray(
        mem.reshape(B, F2, 2, D).transpose(0, 2, 3, 1).reshape(B, P, F2))
    # host-side repack of the tiny replicated weights into SBUF tile layouts
    wk_pack = np.ascontiguousarray(
        np.concatenate([full["Wk"][0:P, :], full["Wk"][P:C, :]], axis=1))
    wh = np.concatenate(
        [full["Wb"], full["Wgate"], full["Ws"], full["Wg"]], axis=1)  # [C, 6]
    wh_pack = np.ascontiguousarray(np.concatenate([wh[0:P], wh[P:C]], axis=1))
    b_pack = np.ascontiguousarray(np.concatenate(
        [full["bb"].reshape(-1), full["bgate"].reshape(-1),
         full["bs"].reshape(-1), full["bg"].reshape(-1)]).reshape(1, 6))
    in_maps = []
    for c in range(NCORES):
        sl = slice(c * BL, (c + 1) * BL)
        in_maps.append({
            "memT": memT[sl],
            "controller_state": full["controller_state"][sl],
            "previous_weights": full["previous_weights"][sl],
            "wk_pack": wk_pack, "wh_pack": wh_pack, "b_pack": b_pack,
        })
    return in_maps


def run(inputs, **kwargs):
    from concourse.bass_utils import run_bass_kernel_spmd
    nc = _get_nc()
    res = run_bass_kernel_spmd(nc, _make_in_maps(inputs),
                               list(range(NCORES)), **kwargs)
    out = np.concatenate([res.results[c]["out"] for c in range(NCORES)], axis=0)
    return out.astype(np.float32), res


def kernel(**inputs):
    out, _ = run(inputs)
    return out
